# revision 4
# baseline (speedup 1.0000x reference)
"""MixedOperation (FBNet/DARTS moe_routing) Trainium2 kernel.

Math: output = sum_i m_i * (conv_i(x) + b_i) with m = gumbel-softmax(thetas).
The weighted sum of convs is linear in the weights, so all 8 candidate convs
(k = 1,1,3,3,5,5,7,7, SAME, stride 1) collapse into ONE effective 7x7 conv:
    W_eff = sum_i m_i * pad7(W_i),   b_eff = sum_i m_i * b_i
which cuts the FLOPs from sum(k^2)=168 to 49 tap-units (3.4x), then the
single conv is computed as 49 shifted [128x128] @ [128x512] matmuls per
output chunk, batch-sharded over 8 NeuronCores (2 images per core).
"""

import numpy as np

_NC = 8
_B, _C, _H, _W = 16, 128, 32, 32
_BPC = _B // _NC  # images per core
_HP = _H + 6      # padded spatial
_KK = 7

_nc_cache = None
_last_in_maps = None


def _build_kernel(mm_dtype_name=None):
    import os

    if mm_dtype_name is None:
        mm_dtype_name = os.environ.get("MM_DT", "float32r")
    import concourse.mybir as mybir
    import concourse.tile as tile
    from concourse import bacc

    f32 = mybir.dt.float32
    mm_dt = getattr(mybir.dt, mm_dtype_name)

    nc = bacc.Bacc("TRN2", target_bir_lowering=False, debug=False, num_devices=_NC)
    xp = nc.dram_tensor("xp", [_C, _BPC, _HP, _HP], f32, kind="ExternalInput").ap()
    wt = nc.dram_tensor("wt", [_C, _KK, _KK, _C], f32, kind="ExternalInput").ap()
    bb = nc.dram_tensor("bb", [_C, 1], f32, kind="ExternalInput").ap()
    y = nc.dram_tensor("y", [_C, _BPC, _H, _W], f32, kind="ExternalOutput").ap()

    with tile.TileContext(nc) as tc:
        with (
            tc.tile_pool(name="xpool", bufs=1) as xpool,
            tc.tile_pool(name="wpool", bufs=1) as wpool,
            tc.tile_pool(name="bpool", bufs=1) as bpool,
            tc.tile_pool(name="pspool", bufs=4, space="PSUM") as pspool,
            tc.tile_pool(name="opool", bufs=3) as opool,
        ):
            b_sb = bpool.tile([_C, 1], f32, tag="bias")
            nc.sync.dma_start(out=b_sb[:], in_=bb[:])
            x_sb = []
            for b in range(_BPC):
                t = xpool.tile([_C, _HP, _HP], f32, tag=f"x{b}")
                nc.sync.dma_start(out=t[:], in_=xp[:, b])
                x_sb.append(t)
            w_sb = []
            for ky in range(_KK):
                t = wpool.tile([_C, _KK, _C], f32, tag=f"w{ky}")
                nc.sync.dma_start(out=t[:], in_=wt[:, ky])
                w_sb.append(t)

            for b in range(_BPC):
                for yh in range(2):  # 16-row output chunks -> free dim 512
                    ps = pspool.tile([_C, 16, _W], f32, tag="ps")
                    for ky in range(_KK):
                        for kx in range(_KK):
                            rhs = x_sb[b][
                                :, yh * 16 + ky : yh * 16 + ky + 16, kx : kx + _W
                            ]
                            lhsT = w_sb[ky][:, kx, :]
                            nc.tensor.matmul(
                                ps[:],
                                lhsT.bitcast(mm_dt),
                                rhs.bitcast(mm_dt),
                                start=(ky == 0 and kx == 0),
                                stop=(ky == _KK - 1 and kx == _KK - 1),
                            )
                    o = opool.tile([_C, 16, _W], f32, tag="o")
                    nc.scalar.activation(
                        o[:],
                        ps[:],
                        mybir.ActivationFunctionType.Identity,
                        bias=b_sb[:, 0:1],
                        scale=1.0,
                    )
                    nc.sync.dma_start(out=y[:, b, yh * 16 : (yh + 1) * 16, :], in_=o[:])

    nc.compile()
    return nc


def kernel(x, temperature, flops_to_accumulate, params_to_accumulate,
           thetas, gumbel_noise, flops_c, params_c, w_k1, w_k3, w_k5, w_k7, b):
    global _nc_cache, _last_in_maps
    from concourse.bass_utils import run_bass_kernel_spmd

    x = np.asarray(x, np.float32)
    thetas = np.asarray(thetas, np.float32)
    gumbel_noise = np.asarray(gumbel_noise, np.float32)
    flops_c = np.asarray(flops_c, np.float32)
    params_c = np.asarray(params_c, np.float32)
    b = np.asarray(b, np.float32)

    # m = softmax((log_softmax(thetas) + gumbel) / tau), all in f32 like jax
    ls = thetas - (np.max(thetas) + np.log(np.sum(np.exp(thetas - np.max(thetas)))))
    logits = (ls + gumbel_noise) / np.float32(temperature)
    e = np.exp(logits - np.max(logits))
    m = e / np.sum(e)

    ws = [np.asarray(w, np.float32)[j] for w in (w_k1, w_k3, w_k5, w_k7) for j in (0, 1)]
    W = np.zeros((_C, _C, _KK, _KK), np.float32)
    for i, w in enumerate(ws):
        k = w.shape[-1]
        o = (_KK - k) // 2
        W[:, :, o : o + k, o : o + k] += m[i] * w
    b_eff = (m[:, None] * b).sum(axis=0).astype(np.float32)

    # device layouts: wt[ci, ky, kx, co]; x padded+channel-major per core
    wt = np.ascontiguousarray(np.transpose(W, (1, 2, 3, 0)))
    xpad = np.pad(x, ((0, 0), (0, 0), (3, 3), (3, 3)))
    xpt = np.transpose(xpad, (1, 0, 2, 3))  # [C, B, HP, HP]

    if _nc_cache is None:
        _nc_cache = _build_kernel()

    bb = np.ascontiguousarray(b_eff.reshape(_C, 1))
    in_maps = [
        {
            "xp": np.ascontiguousarray(xpt[:, _BPC * c : _BPC * (c + 1)]),
            "wt": wt,
            "bb": bb,
        }
        for c in range(_NC)
    ]
    _last_in_maps = in_maps
    res = run_bass_kernel_spmd(_nc_cache, in_maps, list(range(_NC)))
    output = np.concatenate(
        [np.transpose(res.results[c]["y"], (1, 0, 2, 3)) for c in range(_NC)], axis=0
    )

    flops_acc = (np.float32(flops_to_accumulate) + np.dot(m, flops_c)).astype(np.float32)
    params_acc = (np.float32(params_to_accumulate) + np.dot(m, params_c)).astype(np.float32)
    return output, flops_acc, params_acc


# revision 7
# speedup vs baseline: 2.6689x; 2.6689x over previous
"""MixedOperation (FBNet/DARTS moe_routing) Trainium2 kernel.

Math: output = sum_i m_i * (conv_i(x) + b_i) with m = gumbel-softmax(thetas).
The weighted sum of convs is linear in the weights, so all 8 candidate convs
(k = 1,1,3,3,5,5,7,7, SAME, stride 1) collapse into ONE effective 7x7 conv:
    W_eff = sum_i m_i * pad7(W_i),   b_eff = sum_i m_i * b_i
which cuts the FLOPs from sum(k^2)=168 to 49 tap-units (3.4x), then the
single conv is computed as 49 shifted [128x128] @ [128x512] matmuls per
output chunk, batch-sharded over 8 NeuronCores (2 images per core).
"""

import numpy as np

_NC = 8
_B, _C, _H, _W = 16, 128, 32, 32
_BPC = _B // _NC  # images per core
_HP = _H + 6      # padded spatial
_KK = 7

_nc_cache = None
_last_in_maps = None


def _build_kernel(mm_dtype_name=None):
    import os

    if mm_dtype_name is None:
        mm_dtype_name = os.environ.get("MM_DT", "float32r")
    import concourse.mybir as mybir
    import concourse.tile as tile
    from concourse import bacc

    f32 = mybir.dt.float32
    mm_dt = getattr(mybir.dt, mm_dtype_name)

    nc = bacc.Bacc("TRN2", target_bir_lowering=False, debug=False, num_devices=_NC)
    xp = nc.dram_tensor("xp", [_C, _BPC, _HP, _HP], mm_dt, kind="ExternalInput").ap()
    wt = nc.dram_tensor("wt", [_C, _KK, _KK, _C], mm_dt, kind="ExternalInput").ap()
    bb = nc.dram_tensor("bb", [_C, 1], f32, kind="ExternalInput").ap()
    y = nc.dram_tensor("y", [_C, _BPC, _H, _W], f32, kind="ExternalOutput").ap()

    with tile.TileContext(nc) as tc:
        with (
            tc.tile_pool(name="xpool", bufs=1) as xpool,
            tc.tile_pool(name="wpool", bufs=1) as wpool,
            tc.tile_pool(name="bpool", bufs=1) as bpool,
            tc.tile_pool(name="pspool", bufs=4, space="PSUM") as pspool,
            tc.tile_pool(name="opool", bufs=3) as opool,
        ):
            b_sb = bpool.tile([_C, 1], f32, tag="bias")
            nc.sync.dma_start(out=b_sb[:], in_=bb[:])
            x_sb = []
            for b in range(_BPC):
                t = xpool.tile([_C, _HP, _HP], mm_dt, tag=f"x{b}")
                nc.sync.dma_start(out=t[:], in_=xp[:, b])
                x_sb.append(t)
            w_sb = []
            for ky in range(_KK):
                t = wpool.tile([_C, _KK, _C], mm_dt, tag=f"w{ky}")
                nc.sync.dma_start(out=t[:], in_=wt[:, ky])
                w_sb.append(t)

            for b in range(_BPC):
                for yh in range(2):  # 16-row output chunks -> free dim 512
                    ps = pspool.tile([_C, 16, _W], f32, tag="ps")
                    for ky in range(_KK):
                        for kx in range(_KK):
                            rhs = x_sb[b][
                                :, yh * 16 + ky : yh * 16 + ky + 16, kx : kx + _W
                            ]
                            lhsT = w_sb[ky][:, kx, :]
                            nc.tensor.matmul(
                                ps[:],
                                lhsT,
                                rhs,
                                start=(ky == 0 and kx == 0),
                                stop=(ky == _KK - 1 and kx == _KK - 1),
                            )
                    o = opool.tile([_C, 16, _W], f32, tag="o")
                    nc.scalar.activation(
                        o[:],
                        ps[:],
                        mybir.ActivationFunctionType.Identity,
                        bias=b_sb[:, 0:1],
                        scale=1.0,
                    )
                    nc.sync.dma_start(out=y[:, b, yh * 16 : (yh + 1) * 16, :], in_=o[:])

    nc.compile()
    return nc


def kernel(x, temperature, flops_to_accumulate, params_to_accumulate,
           thetas, gumbel_noise, flops_c, params_c, w_k1, w_k3, w_k5, w_k7, b):
    global _nc_cache, _last_in_maps
    from concourse.bass_utils import run_bass_kernel_spmd

    x = np.asarray(x, np.float32)
    thetas = np.asarray(thetas, np.float32)
    gumbel_noise = np.asarray(gumbel_noise, np.float32)
    flops_c = np.asarray(flops_c, np.float32)
    params_c = np.asarray(params_c, np.float32)
    b = np.asarray(b, np.float32)

    # m = softmax((log_softmax(thetas) + gumbel) / tau), all in f32 like jax
    ls = thetas - (np.max(thetas) + np.log(np.sum(np.exp(thetas - np.max(thetas)))))
    logits = (ls + gumbel_noise) / np.float32(temperature)
    e = np.exp(logits - np.max(logits))
    m = e / np.sum(e)

    ws = [np.asarray(w, np.float32)[j] for w in (w_k1, w_k3, w_k5, w_k7) for j in (0, 1)]
    W = np.zeros((_C, _C, _KK, _KK), np.float32)
    for i, w in enumerate(ws):
        k = w.shape[-1]
        o = (_KK - k) // 2
        W[:, :, o : o + k, o : o + k] += m[i] * w
    b_eff = (m[:, None] * b).sum(axis=0).astype(np.float32)

    # device layouts: wt[ci, ky, kx, co]; x padded+channel-major per core
    wt = np.ascontiguousarray(np.transpose(W, (1, 2, 3, 0)))
    xpad = np.pad(x, ((0, 0), (0, 0), (3, 3), (3, 3)))
    xpt = np.transpose(xpad, (1, 0, 2, 3))  # [C, B, HP, HP]

    if _nc_cache is None:
        _nc_cache = _build_kernel()

    bb = np.ascontiguousarray(b_eff.reshape(_C, 1))
    in_maps = [
        {
            "xp": np.ascontiguousarray(xpt[:, _BPC * c : _BPC * (c + 1)]),
            "wt": wt,
            "bb": bb,
        }
        for c in range(_NC)
    ]
    _last_in_maps = in_maps
    res = run_bass_kernel_spmd(_nc_cache, in_maps, list(range(_NC)))
    output = np.concatenate(
        [np.transpose(res.results[c]["y"], (1, 0, 2, 3)) for c in range(_NC)], axis=0
    )

    flops_acc = (np.float32(flops_to_accumulate) + np.dot(m, flops_c)).astype(np.float32)
    params_acc = (np.float32(params_to_accumulate) + np.dot(m, params_c)).astype(np.float32)
    return output, flops_acc, params_acc


# revision 8
# speedup vs baseline: 2.9685x; 1.1123x over previous
"""MixedOperation (FBNet/DARTS moe_routing) Trainium2 kernel.

Math: output = sum_i m_i * (conv_i(x) + b_i) with m = gumbel-softmax(thetas).
The weighted sum of convs is linear in the weights, so all 8 candidate convs
(k = 1,1,3,3,5,5,7,7, SAME, stride 1) collapse into ONE effective 7x7 conv:
    W_eff = sum_i m_i * pad7(W_i),   b_eff = sum_i m_i * b_i
which cuts the FLOPs from sum(k^2)=168 to 49 tap-units (3.4x), then the
single conv is computed as 49 shifted [128x128] @ [128x512] matmuls per
output chunk, batch-sharded over 8 NeuronCores (2 images per core).
"""

import os

import numpy as np

_NC = 8
_B, _C, _H, _W = 16, 128, 32, 32
_BPC = _B // _NC  # images per core
_HP = _H + 6      # padded spatial
_KK = 7
_NWARM = int(os.environ.get("NWARM", "28"))  # PE warmup matmuls

_nc_cache = None
_last_in_maps = None


def _build_kernel(mm_dtype_name=None):
    import concourse.mybir as mybir
    import concourse.tile as tile
    from concourse import bacc

    if mm_dtype_name is None:
        mm_dtype_name = os.environ.get("MM_DT", "float16")

    f32 = mybir.dt.float32
    mm_dt = getattr(mybir.dt, mm_dtype_name)

    nc = bacc.Bacc("TRN2", target_bir_lowering=False, debug=False, num_devices=_NC)
    xp = nc.dram_tensor("xp", [_C, _BPC, _HP, _HP], mm_dt, kind="ExternalInput").ap()
    wt = nc.dram_tensor("wt", [_C, _KK, _KK, _C], mm_dt, kind="ExternalInput").ap()
    bb = nc.dram_tensor("bb", [_C, 1], f32, kind="ExternalInput").ap()
    y = nc.dram_tensor("y", [_C, _BPC, _H, _W], f32, kind="ExternalOutput").ap()

    with tile.TileContext(nc) as tc:
        with (
            tc.tile_pool(name="xpool", bufs=1) as xpool,
            tc.tile_pool(name="wpool", bufs=1) as wpool,
            tc.tile_pool(name="bpool", bufs=1) as bpool,
            tc.tile_pool(name="pspool", bufs=4, space="PSUM") as pspool,
            tc.tile_pool(name="opool", bufs=3) as opool,
        ):
            # zero tiles for PE warmup (keeps HAM at K=8/8 during input DMA)
            zw = bpool.tile([_C, _C], mm_dt, tag="zw")
            zx = bpool.tile([_C, 512], mm_dt, tag="zx")
            nc.gpsimd.memset(zw[:], 0)
            nc.gpsimd.memset(zx[:], 0)

            b_sb = bpool.tile([_C, 1], f32, tag="bias")
            nc.sync.dma_start(out=b_sb[:], in_=bb[:])
            x_sb = []
            for b in range(_BPC):
                t = xpool.tile([_C, _HP, _HP], mm_dt, tag=f"x{b}")
                nc.sync.dma_start(out=t[:], in_=xp[:, b])
                x_sb.append(t)
            w_sb = []
            for ky in range(_KK):
                t = wpool.tile([_C, _KK, _C], mm_dt, tag=f"w{ky}")
                nc.gpsimd.dma_start(out=t[:], in_=wt[:, ky])
                w_sb.append(t)

            for b in range(_BPC):
                for yh in range(2):  # 16-row output chunks -> free dim 512
                    ps = pspool.tile([_C, 16, _W], f32, tag="ps")
                    first_chunk = b == 0 and yh == 0
                    if first_chunk:
                        # warmup: accumulate zeros; warms the PE clock while
                        # the real input DMAs are still in flight
                        for i in range(_NWARM):
                            nc.tensor.matmul(
                                ps[:], zw[:], zx[:, :512].rearrange("p (a b) -> p a b", a=16),
                                start=(i == 0), stop=False,
                            )
                    for ky in range(_KK):
                        for kx in range(_KK):
                            rhs = x_sb[b][
                                :, yh * 16 + ky : yh * 16 + ky + 16, kx : kx + _W
                            ]
                            lhsT = w_sb[ky][:, kx, :]
                            nc.tensor.matmul(
                                ps[:],
                                lhsT,
                                rhs,
                                start=(not first_chunk) and (ky == 0 and kx == 0),
                                stop=(ky == _KK - 1 and kx == _KK - 1),
                            )
                    o = opool.tile([_C, 16, _W], f32, tag="o")
                    nc.vector.tensor_scalar_add(o[:], ps[:], b_sb[:, 0:1])
                    nc.sync.dma_start(out=y[:, b, yh * 16 : (yh + 1) * 16, :], in_=o[:])

    nc.compile()
    return nc


def kernel(x, temperature, flops_to_accumulate, params_to_accumulate,
           thetas, gumbel_noise, flops_c, params_c, w_k1, w_k3, w_k5, w_k7, b):
    global _nc_cache, _last_in_maps
    from concourse.bass_utils import run_bass_kernel_spmd

    x = np.asarray(x, np.float32)
    thetas = np.asarray(thetas, np.float32)
    gumbel_noise = np.asarray(gumbel_noise, np.float32)
    flops_c = np.asarray(flops_c, np.float32)
    params_c = np.asarray(params_c, np.float32)
    b = np.asarray(b, np.float32)

    # m = softmax((log_softmax(thetas) + gumbel) / tau), all in f32 like jax
    ls = thetas - (np.max(thetas) + np.log(np.sum(np.exp(thetas - np.max(thetas)))))
    logits = (ls + gumbel_noise) / np.float32(temperature)
    e = np.exp(logits - np.max(logits))
    m = e / np.sum(e)

    ws = [np.asarray(w, np.float32)[j] for w in (w_k1, w_k3, w_k5, w_k7) for j in (0, 1)]
    W = np.zeros((_C, _C, _KK, _KK), np.float32)
    for i, w in enumerate(ws):
        k = w.shape[-1]
        o = (_KK - k) // 2
        W[:, :, o : o + k, o : o + k] += m[i] * w
    b_eff = (m[:, None] * b).sum(axis=0).astype(np.float32)

    mm_np = np.float16 if os.environ.get("MM_DT", "float16") == "float16" else np.float32

    # device layouts: wt[ci, ky, kx, co]; x padded+channel-major per core
    wt = np.ascontiguousarray(np.transpose(W, (1, 2, 3, 0))).astype(mm_np)
    xpad = np.pad(x, ((0, 0), (0, 0), (3, 3), (3, 3))).astype(mm_np)
    xpt = np.transpose(xpad, (1, 0, 2, 3))  # [C, B, HP, HP]

    if _nc_cache is None:
        _nc_cache = _build_kernel()

    bb = np.ascontiguousarray(b_eff.reshape(_C, 1))
    in_maps = [
        {
            "xp": np.ascontiguousarray(xpt[:, _BPC * c : _BPC * (c + 1)]),
            "wt": wt,
            "bb": bb,
        }
        for c in range(_NC)
    ]
    _last_in_maps = in_maps
    res = run_bass_kernel_spmd(_nc_cache, in_maps, list(range(_NC)))
    output = np.concatenate(
        [np.transpose(res.results[c]["y"], (1, 0, 2, 3)) for c in range(_NC)], axis=0
    )

    flops_acc = (np.float32(flops_to_accumulate) + np.dot(m, flops_c)).astype(np.float32)
    params_acc = (np.float32(params_to_accumulate) + np.dot(m, params_c)).astype(np.float32)
    return output, flops_acc, params_acc


# revision 11
# speedup vs baseline: 3.1989x; 1.0776x over previous
"""MixedOperation (FBNet/DARTS moe_routing) Trainium2 kernel.

Math: output = sum_i m_i * (conv_i(x) + b_i) with m = gumbel-softmax(thetas).
The weighted sum of convs is linear in the weights, so all 8 candidate convs
(k = 1,1,3,3,5,5,7,7, SAME, stride 1) collapse into ONE effective 7x7 conv:
    W_eff = sum_i m_i * pad7(W_i),   b_eff = sum_i m_i * b_i
which cuts the FLOPs from sum(k^2)=168 to 49 tap-units (3.4x). The single
conv runs as 49 shifted [128x128] @ [128x512] fp16 matmuls per output chunk
(fp32 PSUM accumulate), batch-sharded over 8 NeuronCores (2 images/core).

Raw bacc kernel (no Tile framework): hand-rolled semaphores, dedicated PSUM
bank per output chunk, PE warmup matmuls on zeros during the input DMAs.
"""

import os

import numpy as np

_NC = 8
_B, _C, _H, _W = 16, 128, 32, 32
_BPC = _B // _NC  # images per core
_HP = _H + 6      # padded spatial
_KK = 7
_NWARM = int(os.environ.get("NWARM", "8"))

_nc_cache = None
_last_in_maps = None


def _build_kernel():
    import concourse.mybir as mybir
    from concourse import bacc

    mm_dt = getattr(mybir.dt, os.environ.get("MM_DT", "float16"))
    f32 = mybir.dt.float32

    nc = bacc.Bacc("TRN2", target_bir_lowering=False, debug=False, num_devices=_NC)
    xp = nc.dram_tensor("xp", [_C, _BPC, _HP, _HP], mm_dt, kind="ExternalInput").ap()
    wt = nc.dram_tensor("wt", [_C, _KK, _KK, _C], mm_dt, kind="ExternalInput").ap()
    bb = nc.dram_tensor("bb", [_C, 1], f32, kind="ExternalInput").ap()
    y = nc.dram_tensor("y", [_C, _BPC, _H, _W], f32, kind="ExternalOutput").ap()

    # chunk c -> (image b, row-half yh); each chunk = [128 c_out, 16 rows, 32 cols]
    chunks = [(b, yh) for b in range(_BPC) for yh in range(2)]

    from contextlib import ExitStack

    with ExitStack() as ctx:
        x_sb = [
            ctx.enter_context(nc.sbuf_tensor(f"x{i}", [_C, _HP, _HP], mm_dt))
            for i in range(_BPC)
        ]
        w_sb = ctx.enter_context(nc.sbuf_tensor("w_sb", [_C, _KK, _KK, _C], mm_dt))
        b_sb = ctx.enter_context(nc.sbuf_tensor("b_sb", [_C, 1], f32))
        zw = ctx.enter_context(nc.sbuf_tensor("zw", [_C, _C], mm_dt))
        zx = ctx.enter_context(nc.sbuf_tensor("zx", [_C, 512], mm_dt))
        o_sb = [
            ctx.enter_context(nc.sbuf_tensor(f"o{i}", [_C, 16, _W], f32))
            for i in range(4)
        ]
        ps = [
            ctx.enter_context(nc.psum_tensor(f"ps{i}", [_C, 16, _W], f32))
            for i in range(4)
        ]
        s_x = ctx.enter_context(nc.semaphore("s_x"))
        s_w = ctx.enter_context(nc.semaphore("s_w"))
        s_b = ctx.enter_context(nc.semaphore("s_b"))
        s_z = ctx.enter_context(nc.semaphore("s_z"))
        s_mm = ctx.enter_context(nc.semaphore("s_mm"))
        s_v = ctx.enter_context(nc.semaphore("s_v"))
        s_out = ctx.enter_context(nc.semaphore("s_out"))
        block = ctx.enter_context(nc.Block())
        x0 = x_sb[0]
        x1 = x_sb[-1]

        @block.gpsimd
        def _(gpsimd):
            gpsimd.memset(zw[:], 0).then_inc(s_z, 1)
            gpsimd.memset(zx[:], 0).then_inc(s_z, 1)
            for ky in range(_KK):
                gpsimd.dma_start(out=w_sb[:, ky], in_=wt[:, ky]).then_inc(s_w, 16)

        @block.sync
        def _(sync):
            # first 22 rows of image 0 unblock chunk 0
            sync.dma_start(out=x0[:, 0:22, :], in_=xp[:, 0, 0:22, :]).then_inc(s_x, 16)
            sync.dma_start(out=x0[:, 22:, :], in_=xp[:, 0, 22:, :]).then_inc(s_x, 16)
            if _BPC > 1:
                sync.dma_start(out=x1[:], in_=xp[:, 1]).then_inc(s_x, 16)
            sync.dma_start(out=b_sb[:], in_=bb[:]).then_inc(s_b, 16)
            for c, (b, yh) in enumerate(chunks):
                sync.wait_ge(s_v, c + 1)
                sync.dma_start(
                    out=y[:, b, yh * 16 : (yh + 1) * 16, :], in_=o_sb[c][:]
                ).then_inc(s_out, 16)
            sync.wait_ge(s_out, 16 * len(chunks))

        @block.tensor
        def _(tensor):
            tensor.wait_ge(s_z, 2)
            for i in range(_NWARM):
                tensor.matmul(ps[0][:], zw[:], zx[:], start=(i == 0), stop=False)
            for c, (b, yh) in enumerate(chunks):
                # x rows needed: chunk0 -> first 22 rows; chunk1 -> rest of
                # image 0; chunks 2,3 -> image 1
                tensor.wait_ge(s_x, 16 * (1 if c == 0 else 2 if c == 1 else 3))
                if c > 0:
                    tensor.wait_ge(s_w, 16 * _KK)
                for ky in range(_KK):
                    if c == 0:
                        tensor.wait_ge(s_w, 16 * (ky + 1))
                    for kx in range(_KK):
                        rhs = x_sb[b][
                            :, yh * 16 + ky : yh * 16 + ky + 16, kx : kx + _W
                        ]
                        ins = tensor.matmul(
                            ps[c][:],
                            w_sb[:, ky, kx, :],
                            rhs,
                            start=(c > 0 and ky == 0 and kx == 0),
                            stop=(ky == _KK - 1 and kx == _KK - 1),
                        )
                ins.then_inc(s_mm, 1)

        @block.vector
        def _(vector):
            vector.wait_ge(s_b, 16)
            for c in range(len(chunks)):
                vector.wait_ge(s_mm, c + 1)
                vector.tensor_scalar_add(o_sb[c][:], ps[c][:], b_sb[:, 0:1]).then_inc(
                    s_v, 1
                )

    nc.compile()
    return nc


def kernel(x, temperature, flops_to_accumulate, params_to_accumulate,
           thetas, gumbel_noise, flops_c, params_c, w_k1, w_k3, w_k5, w_k7, b):
    global _nc_cache, _last_in_maps
    from concourse.bass_utils import run_bass_kernel_spmd

    x = np.asarray(x, np.float32)
    thetas = np.asarray(thetas, np.float32)
    gumbel_noise = np.asarray(gumbel_noise, np.float32)
    flops_c = np.asarray(flops_c, np.float32)
    params_c = np.asarray(params_c, np.float32)
    b = np.asarray(b, np.float32)

    # m = softmax((log_softmax(thetas) + gumbel) / tau), all in f32 like jax
    ls = thetas - (np.max(thetas) + np.log(np.sum(np.exp(thetas - np.max(thetas)))))
    logits = (ls + gumbel_noise) / np.float32(temperature)
    e = np.exp(logits - np.max(logits))
    m = e / np.sum(e)

    ws = [np.asarray(w, np.float32)[j] for w in (w_k1, w_k3, w_k5, w_k7) for j in (0, 1)]
    W = np.zeros((_C, _C, _KK, _KK), np.float32)
    for i, w in enumerate(ws):
        k = w.shape[-1]
        o = (_KK - k) // 2
        W[:, :, o : o + k, o : o + k] += m[i] * w
    b_eff = (m[:, None] * b).sum(axis=0).astype(np.float32)

    mm_np = np.float16 if os.environ.get("MM_DT", "float16") == "float16" else np.float32

    # device layouts: wt[ci, ky, kx, co]; x padded+channel-major per core
    wt = np.ascontiguousarray(np.transpose(W, (1, 2, 3, 0))).astype(mm_np)
    xpad = np.pad(x, ((0, 0), (0, 0), (3, 3), (3, 3))).astype(mm_np)
    xpt = np.transpose(xpad, (1, 0, 2, 3))  # [C, B, HP, HP]

    if _nc_cache is None:
        _nc_cache = _build_kernel()

    bb = np.ascontiguousarray(b_eff.reshape(_C, 1))
    in_maps = [
        {
            "xp": np.ascontiguousarray(xpt[:, _BPC * c : _BPC * (c + 1)]),
            "wt": wt,
            "bb": bb,
        }
        for c in range(_NC)
    ]
    _last_in_maps = in_maps
    res = run_bass_kernel_spmd(_nc_cache, in_maps, list(range(_NC)))
    output = np.concatenate(
        [np.transpose(res.results[c]["y"], (1, 0, 2, 3)) for c in range(_NC)], axis=0
    )

    flops_acc = (np.float32(flops_to_accumulate) + np.dot(m, flops_c)).astype(np.float32)
    params_acc = (np.float32(params_to_accumulate) + np.dot(m, params_c)).astype(np.float32)
    return output, flops_acc, params_acc
